# revision 23
# baseline (speedup 1.0000x reference)
"""Trainium2 Bass kernel for nn_BinaryMLP (BitNet-ternary SwiGLU MLP).

reference math (fp32):
    s_i = mean(|w_i|)            (per-tensor scalar, i in {1,3,2})
    wq_i = clip(round(w_i/s_i), -1, 1) * s_i     (ternary * scale)
    h1 = x @ w1q.T ; h3 = x @ w3q.T
    y  = (silu(h1) * h3) @ w2q.T

Strategy (8 cores, data-parallel over the 16384 tokens):
  - host: pad H 5461->5504, transpose x / w1 / w3 / w2 into contraction-major
    layouts (pure layout work, no arithmetic), split tokens 8 ways, and give
    each core a distinct 1/8 row-slice of each weight tensor.
  - device (per core, identical SPMD program):
      phase A (head, w1/w3 only): |w1| partial sums on DVE while |w3| runs
               on GpSimd (both DMA-paced) -> partition_all_reduce -> tiny
               8-core AllGather of the per-core sums -> on-core global sum
               -> ternarization thresholds +-s/2 as per-partition biases.
               w2's scale pass is NOT on the critical path: s2 is factored
               out of phase B (g' = silu(h1)*h3 carries no s2; phase C's
               PSUM->SBUF copy applies s2/2), so w2's |w| pass, thresholds,
               ternarize and AllGather all run in the middle of phase B.
      phase B: h1/h3 matmuls vs resident bf16 x (fp8 ternary weights
               stationary, 8 psum banks), g' = silu((s1/2) z1) *
               ((s3/2) z3) -> bf16 -> DRAM.  The shard-ternarize of weight
               chunk q+1 and its AllGather are emitted interleaved INSIDE
               phase B's chunk-q compute so the DMA trigger queues never
               head-of-line-block phase B's weight loads.  Half of w2 is
               prefetched to SBUF (via the GpSimd DMA queue) before phase C.
      phase C: y[m,d] = (s2/2) * sum_h g'[h,m] t2[h,d], g' stationary, fp32
               PSUM, d in two halves so half of w2 is SBUF-resident before
               the phase starts (kills the B->C bubble); the second half
               streams in during the first half's matmuls.
  - host: concatenate the 8 token shards, reshape to [4, 4096, 2048].

All arithmetic (scales, ternarization, matmuls) happens on device; the host
only reshapes / transposes / pads / slices / concatenates.
"""

import sys
from contextlib import ExitStack

import numpy as np

if "/opt/trn_rl_repo" not in sys.path:
    sys.path.insert(0, "/opt/trn_rl_repo")

import concourse.bass as bass  # noqa: E402,F401
import concourse.mybir as mybir  # noqa: E402
import concourse.tile as tile  # noqa: E402
from concourse import bacc  # noqa: E402
from concourse.bass_isa import ReduceOp  # noqa: E402

F32 = mybir.dt.float32
BF16 = mybir.dt.bfloat16
FP8 = mybir.dt.float8e4
AF = mybir.ActivationFunctionType
ALU = mybir.AluOpType
AX = mybir.AxisListType

# Full problem geometry (hardcoded per contest rules).
B, S, D = 4, 4096, 2048
H_REAL = 5461
HP = 5504            # H padded to 43*128
N_CORES = 8
M = (B * S) // N_CORES   # tokens per core = 2048


def build_module(d=D, m=M, hp=HP, n_cores=N_CORES, h_real=H_REAL,
                 hb=4, w13_dt=FP8, w2q_dt=FP8):
    """Build + compile the per-core SPMD Bass module."""
    kd = d // 128        # k-tiles over D
    ht = hp // 128       # h-tiles
    mc = m // 512        # m-chunks of 512 in phase B
    assert d % 128 == 0 and hp % 128 == 0 and m % 512 == 0
    n_true = h_real * d
    sw = d * hp // (n_cores * 128)   # w2 slice free elems per partition
    r13 = d // n_cores               # weight-slice rows (w1t/w3t)
    assert r13 % 128 == 0
    a13 = r13 // 128

    # AllGather chunks over h-tiles: small first chunks for a fast phase-B
    # start, the rest big.
    chunks = []
    t0 = 0
    for n in (4, 4, 6, 8, 21):
        chunks.append((t0, n))
        t0 += n
    assert t0 == ht
    nq = len(chunks)

    nc = bacc.Bacc(
        "TRN2",
        target_bir_lowering=False,
        debug=False,
        num_devices=n_cores,
    )
    xT = nc.dram_tensor("xT", [d, m], F32, kind="ExternalInput").ap()
    wsh1 = nc.dram_tensor("wsh1", [r13, hp], F32, kind="ExternalInput").ap()
    wsh3 = nc.dram_tensor("wsh3", [r13, hp], F32, kind="ExternalInput").ap()
    wsh2 = nc.dram_tensor("wsh2", [128, sw], F32, kind="ExternalInput").ap()
    y = nc.dram_tensor("y", [m, d], F32, kind="ExternalOutput").ap()

    xview = xT.rearrange("(k p) m -> p k m", p=128)
    v1 = wsh1.rearrange("(a p) h -> p a h", p=128)   # [128, a13, hp]
    v3 = wsh3.rearrange("(a p) h -> p a h", p=128)

    with tile.TileContext(nc) as tc:
        with ExitStack() as ctx:
            dram = ctx.enter_context(tc.tile_pool(name="dram", bufs=1, space="DRAM"))
            g_dram = dram.tile([hp, m], BF16, tag="g", name="g")
            g_rd = g_dram.rearrange("(k p) m -> p k m", p=128)
            # scale-exchange buffers (w13 early, w2 deferred)
            sc13_in = dram.tile([1, 8], F32, tag="sc13i", name="sc13i")
            sc13_out = dram.tile([8, 8], F32, tag="sc13o", name="sc13o")
            sc2_in = dram.tile([1, 8], F32, tag="sc2i", name="sc2i")
            sc2_out = dram.tile([8, 8], F32, tag="sc2o", name="sc2o")

            agin13 = []
            agout13 = []
            for q, (ct0, cnt) in enumerate(chunks):
                agin13.append(dram.tile(
                    [r13, 2, cnt * 128], w13_dt,
                    tag=f"agi{q}", name=f"agi{q}"))
                agout13.append(dram.tile(
                    [d, 2, cnt * 128], w13_dt,
                    tag=f"ago{q}", name=f"ago{q}"))
            agin2 = dram.tile([128, sw], w2q_dt, tag="agi2", name="agi2")
            agout2 = dram.tile([hp, d], w2q_dt, tag="ago2", name="ago2")
            agout2_rd = agout2.rearrange("(k p) dd -> p k dd", p=128)

            # ---- persistent SBUF: biases, x (bf16), w2 first half ----------
            pc = ctx.enter_context(tc.tile_pool(name="pconst", bufs=1))
            bias = {}
            for name in ("w1", "w3", "w2"):
                for sgn in ("p", "n"):
                    bias[name + sgn] = pc.tile(
                        [128, 1], F32, tag=f"b_{name}{sgn}",
                        name=f"b_{name}{sgn}")
            xp = ctx.enter_context(tc.tile_pool(name="xp", bufs=1))
            x_sb = xp.tile([128, kd, m], BF16, tag="x_sb", name="x_sb")
            w2a_p = ctx.enter_context(tc.tile_pool(name="w2a", bufs=1))
            w2A = w2a_p.tile([128, ht, 1024], w2q_dt, tag="w2A", name="w2A")

            def global_scale(part_ap, cc_in, cc_out, sc1_pool, tag,
                             dma_eng=None):
                """per-core sums [128, n] -> global sums broadcast [128, 8].

                partition_all_reduce -> AllGather of [1,8] -> on-core
                partition sum of the 8 rank rows -> broadcast to 128.
                dma_eng: engine whose queue carries the tiny staging DMAs
                (gpsimd for the mid-phase-B variant, so a blocked trigger
                never head-of-line-blocks the Sync queue).
                """
                dma_eng = dma_eng or nc.sync
                par = sc1_pool.tile([128, 8], F32, tag=f"par{tag}",
                                    name=f"par{tag}")
                nc.vector.memset(par, 0.0)
                nw = part_ap.shape[-1]
                nc.vector.tensor_scalar(par[:, 0:nw], part_ap, 1.0, None,
                                        ALU.mult)
                nc.gpsimd.partition_all_reduce(par, par, 128, ReduceOp.add)
                dma_eng.dma_start(cc_in, par[0:1, :])
                nc.gpsimd.collective_compute(
                    "AllGather", ALU.bypass,
                    replica_groups=[list(range(n_cores))],
                    ins=[cc_in.opt()],
                    outs=[cc_out.opt()],
                )
                rk = sc1_pool.tile([8, 8], F32, tag=f"rk{tag}",
                                   name=f"rk{tag}")
                dma_eng.dma_start(rk, cc_out)
                gsum = sc1_pool.tile([8, 8], F32, tag=f"gs{tag}",
                                     name=f"gs{tag}")
                nc.gpsimd.partition_all_reduce(gsum, rk, 8, ReduceOp.add)
                gb = sc1_pool.tile([128, 8], F32, tag=f"gb{tag}",
                                   name=f"gb{tag}")
                nc.gpsimd.partition_broadcast(gb, gsum[0:1, :])
                return gb

            def set_bias(name, src_col):
                for sgn, k in (("p", 0.5 / n_true), ("n", -0.5 / n_true)):
                    nc.vector.tensor_scalar(
                        bias[name + sgn], src_col, k, None, ALU.mult)

            # ------------- phase A: w1/w3 scales (w2 deferred) --------------
            sc1_ctx = ExitStack()
            sc1_pool = sc1_ctx.enter_context(tc.tile_pool(name="sc1", bufs=1))
            with tc.tile_pool(name="scd", bufs=4) as scd_pool, \
                 tc.tile_pool(name="scg", bufs=4) as scg_pool:
                nch = 8
                chunk_h = hp // nch
                asum = sc1_pool.tile([128, 2, nch], F32, tag="asum", name="asum")
                for c in range(nch):
                    hs_ = slice(c * chunk_h, (c + 1) * chunk_h)
                    std = scd_pool.tile([128, a13, chunk_h], F32, tag="scd",
                                        name="scd")
                    nc.sync.dma_start(std, v1[:, :, hs_])
                    nc.vector.tensor_reduce(
                        asum[:, 0, c:c + 1], std, axis=AX.XY, op=ALU.add,
                        apply_absolute_value=True)
                    stg = scg_pool.tile([128, a13, chunk_h], F32, tag="scg",
                                        name="scg")
                    nc.sync.dma_start(stg, v3[:, :, hs_])
                    # |w3| row-sum on ACT (accum_out); the full-size activation
                    # output is scratch.
                    scr = scg_pool.tile([128, a13, chunk_h], FP8, tag="scr",
                                        name="scr")
                    nc.scalar.activation(scr, stg, AF.Abs,
                                         accum_out=asum[:, 1, c:c + 1])
                part2 = sc1_pool.tile([128, 2], F32, tag="part2", name="part2")
                nc.vector.tensor_reduce(part2, asum, axis=AX.X, op=ALU.add)
                gb13 = global_scale(part2, sc13_in, sc13_out, sc1_pool, "13")
                set_bias("w1", gb13[:, 0:1])
                set_bias("w3", gb13[:, 1:2])

            # ------------- phase B (+ interleaved ternarize/AllGather) ------
            with ExitStack() as sb:
                qstg_p = sb.enter_context(tc.tile_pool(name="qstg", bufs=2))
                qq_p = sb.enter_context(tc.tile_pool(name="qq", bufs=2))
                wq_p = sb.enter_context(tc.tile_pool(name="wq", bufs=2))
                sl_p = sb.enter_context(tc.tile_pool(name="slp", bufs=4))
                g_p = sb.enter_context(tc.tile_pool(name="gp", bufs=2))
                zps = sb.enter_context(tc.tile_pool(name="zps", bufs=8, space="PSUM"))

                # x -> bf16 resident; DMA triggers on the ACT queue so their
                # bufs anti-dep chain never paces the Sync queue (which
                # carries phase B's weight-load triggers).  bufs=2 also
                # keeps x's HBM draw low while the scale pass streams w1/w3.
                with tc.tile_pool(name="xstg", bufs=2) as xstg_p:
                    for k in range(kd):
                        xstg = xstg_p.tile([128, m], F32, tag="xstg", name="xstg")
                        nc.scalar.dma_start(xstg, xview[:, k, :])
                        nc.scalar.copy(x_sb[:, k, :], xstg)

                def quantize_dve(out_ap, stg_ap, bn, bp, dt):
                    """DVE-only ternarize: {-2,0,+2} exact in fp8."""
                    pr, fw = stg_ap.shape[0], stg_ap.shape[-1]
                    qa = qq_p.tile([128, fw], dt, tag=f"qa{fw}", name=f"qa{fw}")
                    qb = qq_p.tile([128, fw], dt, tag=f"qb{fw}", name=f"qb{fw}")
                    nc.vector.tensor_scalar(qa[:pr, :fw], stg_ap, bp[:pr],
                                            2.0, ALU.is_ge, ALU.mult)
                    nc.vector.tensor_scalar(qb[:pr, :fw], stg_ap, bn[:pr],
                                            2.0, ALU.is_lt, ALU.mult)
                    nc.vector.tensor_sub(out_ap, qa[:pr, :fw], qb[:pr, :fw])

                def emit_q13_piece(q, a, t, s0, s1):
                    """Ternarize (row-slice, tensor, h-subrange) of w13 chunk q."""
                    ct0, cnt = chunks[q]
                    chw = (s1 - s0) * 128
                    hsl = slice((ct0 + s0) * 128, (ct0 + s1) * 128)
                    rs = slice(a * 128, (a + 1) * 128)
                    wsht = wsh1 if t == 0 else wsh3
                    stg = qstg_p.tile([128, 1024], F32, tag="qstg", name="qstg")
                    nc.sync.dma_start(stg[:, :chw], wsht[rs, hsl])
                    qt = qq_p.tile([128, 1024], w13_dt, tag="qt", name="qt")
                    quantize_dve(qt[:, :chw], stg[:, :chw],
                                 bias["w1n" if t == 0 else "w3n"],
                                 bias["w1p" if t == 0 else "w3p"], w13_dt)
                    nc.sync.dma_start(
                        agin13[q][rs, t, s0 * 128:s1 * 128], qt[:, :chw])

                def emit_ag13(q):
                    nc.gpsimd.collective_compute(
                        "AllGather", ALU.bypass,
                        replica_groups=[list(range(n_cores))],
                        ins=[agin13[q].opt()],
                        outs=[agout13[q].opt()],
                    )

                # deferred w2 scale pass pieces
                nw2 = 32
                cw2 = sw // nw2
                asum2 = sc1_pool.tile([128, nw2], F32, tag="asum2",
                                      name="asum2")

                def emit_sw2_piece(c):
                    stg = qstg_p.tile([128, cw2], F32, tag="sw2", name="sw2")
                    nc.sync.dma_start(stg, wsh2[:, c * cw2:(c + 1) * cw2])
                    nc.vector.tensor_reduce(
                        asum2[:, c:c + 1], stg, axis=AX.XY, op=ALU.add,
                        apply_absolute_value=True)

                def emit_sw2_fin():
                    part1 = sc1_pool.tile([128, 1], F32, tag="p1", name="p1")
                    nc.vector.tensor_reduce(part1, asum2, axis=AX.X, op=ALU.add)
                    gb2 = global_scale(part1, sc2_in, sc2_out, sc1_pool, "2",
                                       dma_eng=nc.gpsimd)
                    set_bias("w2", gb2[:, 0:1])

                n2 = 32
                c2 = sw // n2

                def emit_q2_piece(c):
                    cs = slice(c * c2, (c + 1) * c2)
                    stg = qstg_p.tile([128, c2], F32, tag="q2stg", name="q2stg")
                    nc.sync.dma_start(stg, wsh2[:, cs])
                    qt = qq_p.tile([128, c2], w2q_dt, tag="q2t", name="q2t")
                    quantize_dve(qt, stg, bias["w2n"], bias["w2p"], w2q_dt)
                    nc.sync.dma_start(agin2[:, cs], qt)

                def emit_ag2():
                    nc.gpsimd.collective_compute(
                        "AllGather", ALU.bypass,
                        replica_groups=[list(range(n_cores))],
                        ins=[agin2.opt()],
                        outs=[agout2.opt()],
                    )

                # chunk 0 must precede phase B; the rest interleave into it.
                for a in range(a13):
                    for t in range(2):
                        emit_q13_piece(0, a, t, 0, chunks[0][1])
                emit_ag13(0)

                # short PE warm-up (no data deps beyond x k0)
                warm = pc.tile([128, 128], BF16, tag="warm", name="warm")
                nc.vector.memset(warm, 0.125)
                wz = zps.tile([128, 512], F32, tag="z", name="z")
                NWARM = 12
                for i in range(NWARM):
                    nc.tensor.matmul(wz, lhsT=warm, rhs=x_sb[:, 0, 0:512],
                                     start=(i == 0), stop=(i == NWARM - 1))

                # deferred-work schedule, emitted a few pieces per h-tile.
                pending = []
                for q in range(1, nq):
                    cnt = chunks[q][1]
                    for s0 in range(0, cnt, 8):
                        s1 = min(s0 + 8, cnt)
                        for a in range(a13):
                            for t in range(2):
                                pending.append(("q13", (q, a, t, s0, s1)))
                    pending.append(("ag13", q))
                for c in range(nw2):
                    pending.append(("sw2", c))
                pending.append(("sw2fin", None))
                for c in range(n2):
                    pending.append(("q2", c))
                pending.append(("ag2", None))
                for k2 in range(ht):
                    pending.append(("w2a", k2))
                pend_i = 0

                def drain_pending(n):
                    nonlocal pend_i
                    for _ in range(n):
                        if pend_i >= len(pending):
                            return
                        kind, pl = pending[pend_i]
                        pend_i += 1
                        if kind == "q13":
                            emit_q13_piece(*pl)
                        elif kind == "ag13":
                            emit_ag13(pl)
                        elif kind == "sw2":
                            emit_sw2_piece(pl)
                        elif kind == "sw2fin":
                            emit_sw2_fin()
                        elif kind == "q2":
                            emit_q2_piece(pl)
                        elif kind == "ag2":
                            emit_ag2()
                        elif kind == "w2a":
                            # GpSimd (SWDGE) queue: waits on the w2 AllGather
                            # without head-of-line-blocking phase B's weight
                            # loads on the Sync queue.
                            nc.gpsimd.dma_start(
                                w2A[:, pl, :], agout2_rd[:, pl, 0:1024])

                tile_no = 0
                for q, (ct0, cnt) in enumerate(chunks):
                    agov = agout13[q].rearrange(
                        "(k p) two h -> p k two h", p=128)
                    for b0 in range(0, cnt, hb):
                        nh = min(hb, cnt - b0)
                        hw = nh * 128
                        habs = (ct0 + b0) * 128       # absolute h start
                        wqb = []
                        for t in range(2):
                            wq_t = wq_p.tile([128, kd, hb * 128], w13_dt,
                                             tag=f"wq{t}", name=f"wq{t}")
                            nc.sync.dma_start(
                                wq_t[:, :, :hw],
                                agov[:, :, t, b0 * 128:b0 * 128 + hw])
                            wqb.append(wq_t)

                        for hti in range(nh):
                            # deferred quantize/AG work first: wait-free DVE
                            # ops, so they never sit behind the PSUM-coupled
                            # epilogue in any FIFO.
                            drain_pending(5 if tile_no < 2 else 3)
                            tile_no += 1
                            hs = slice(hti * 128, (hti + 1) * 128)
                            g_t = g_p.tile([128, m], BF16, tag="g_t", name="g_t")
                            zz = [[None] * mc, [None] * mc]
                            for t in range(2):
                                for mci in range(mc):
                                    zz[t][mci] = zps.tile([128, 512], F32,
                                                          tag="z", name="z")
                                for k in range(kd):
                                    for mci in range(mc):
                                        nc.tensor.matmul(
                                            zz[t][mci], lhsT=wqb[t][:, k, hs],
                                            rhs=x_sb[:, k,
                                                     mci * 512:(mci + 1) * 512],
                                            start=(k == 0), stop=(k == kd - 1),
                                        )
                            for mci in range(mc):
                                ms = slice(mci * 512, (mci + 1) * 512)
                                sl = sl_p.tile([128, 512], BF16, tag="sl",
                                               name="sl")
                                nc.scalar.activation(sl, zz[0][mci], AF.Silu,
                                                     bias=0.0, scale=bias["w1p"])
                                sc = sl_p.tile([128, 512], BF16, tag="sc",
                                               name="sc")
                                nc.scalar.activation(sc, zz[1][mci], AF.Copy,
                                                     bias=0.0, scale=bias["w3p"])
                                nc.vector.tensor_mul(g_t[:, ms], sl, sc)
                            nc.sync.dma_start(
                                g_dram[habs + hti * 128:
                                       habs + (hti + 1) * 128, :], g_t)

                drain_pending(len(pending))   # safety: flush leftovers

            sc1_ctx.close()

            # ---------------- phase C (d in two halves) ---------------------
            with ExitStack() as scx:
                w2b_p = scx.enter_context(tc.tile_pool(name="w2b", bufs=1))
                gq_p = scx.enter_context(tc.tile_pool(name="gq", bufs=3))
                y_p = scx.enter_context(tc.tile_pool(name="yp", bufs=4))
                yps = scx.enter_context(
                    tc.tile_pool(name="yps", bufs=8, space="PSUM"))

                # first gq load goes ahead of the w2B streaming so phase C's
                # critical path (last g write -> first gq -> first matmul)
                # isn't delayed by the bulk w2B transfer.
                gq_tiles = {}

                def emit_gq(mt):
                    gq = gq_p.tile([128, ht, 128], BF16, tag="gq", name="gq")
                    nc.sync.dma_start(gq, g_rd[:, :, mt * 128:(mt + 1) * 128])
                    gq_tiles[mt] = gq

                emit_gq(0)
                w2B = w2b_p.tile([128, ht, 1024], w2q_dt, tag="w2B", name="w2B")
                for k2 in range(ht):
                    nc.sync.dma_start(w2B[:, k2, :], agout2_rd[:, k2, 1024:2048])

                w2h = [w2A, w2B]
                for mt in range(m // 128):
                    if mt not in gq_tiles:
                        emit_gq(mt)
                    gq = gq_tiles.pop(mt)
                    for half in range(2):
                        yp2 = [yps.tile([128, 512], F32, tag="yps", name="yps")
                               for _ in range(2)]
                        for k2 in range(ht):
                            for di in range(2):
                                nc.tensor.matmul(
                                    yp2[di],
                                    lhsT=gq[:, k2, :],
                                    rhs=w2h[half][:, k2, di * 512:(di + 1) * 512],
                                    start=(k2 == 0), stop=(k2 == ht - 1),
                                )
                        ysb = y_p.tile([128, 1024], F32, tag="ysb", name="ysb")
                        for di in range(2):
                            nc.scalar.activation(
                                ysb[:, di * 512:(di + 1) * 512], yp2[di],
                                AF.Copy, bias=0.0, scale=bias["w2p"])
                        nc.sync.dma_start(
                            y[mt * 128:(mt + 1) * 128,
                              half * 1024:(half + 1) * 1024], ysb)

    nc.compile()
    return nc


_NC_CACHE = {}


def _get_module():
    if "nc" not in _NC_CACHE:
        _NC_CACHE["nc"] = build_module()
    return _NC_CACHE["nc"]


def prep_inputs(x, w1, w3, w2, d=D, m=M, hp=HP, n_cores=N_CORES):
    """Host-side layout work: pad, transpose, shard, slice. No arithmetic."""
    h_real = w1.shape[0]
    x = np.ascontiguousarray(np.asarray(x, dtype=np.float32))
    xf = x.reshape(-1, d)
    w1t = np.zeros((d, hp), np.float32)
    w1t[:, :h_real] = np.asarray(w1, np.float32).T
    w3t = np.zeros((d, hp), np.float32)
    w3t[:, :h_real] = np.asarray(w3, np.float32).T
    w2t = np.zeros((hp, d), np.float32)
    w2t[:h_real, :] = np.asarray(w2, np.float32).T

    r13 = d // n_cores
    r2 = hp // n_cores
    sw = d * hp // (n_cores * 128)

    in_maps = []
    for c in range(n_cores):
        xc = np.ascontiguousarray(xf[c * m:(c + 1) * m].T)   # [d, m]
        in_maps.append({
            "xT": xc,
            "wsh1": np.ascontiguousarray(w1t[c * r13:(c + 1) * r13]),
            "wsh3": np.ascontiguousarray(w3t[c * r13:(c + 1) * r13]),
            "wsh2": np.ascontiguousarray(
                w2t[c * r2:(c + 1) * r2].reshape(128, sw)),
        })
    return in_maps


def kernel(x, w1, w3, w2):
    from concourse.bass_utils import run_bass_kernel_spmd

    nc = _get_module()
    in_maps = prep_inputs(x, w1, w3, w2)
    res = run_bass_kernel_spmd(nc, in_maps, core_ids=list(range(N_CORES)))
    _NC_CACHE["last_results"] = res
    yf = np.concatenate([r["y"] for r in res.results], axis=0)  # [16384, 2048]
    return np.ascontiguousarray(yf.reshape(B, S, D).astype(np.float32))
